# revision 26
# baseline (speedup 1.0000x reference)
"""Causal self-attention (B=2, S=2048, H=1024, NH=16) on 8 TRN2 NeuronCores.

Sharding: core c handles batch b = c//4 and heads [4*(c%4), 4*(c%4)+4).
Tensor-parallel c_attn (column split); the output projection is computed
from the full head dimension on a 512-row output slice per core, after an
AllToAll inside each 4-core batch group redistributes O^T from head-major
to sequence-major. (AllToAll moves 4x less data than reduce-scattering the
partial c_proj outputs, and cuts per-core c_proj flops 4x.)

Per-core dataflow (all matmuls on the PE array):
  1. x[b] -> x^T via PE transposes (contraction over H needs H on partitions).
  2. QKV: Q^T,K^T [dk, S] and V [S, dk] for its 4 heads (Wq pre-scaled 1/8).
  3. Attention per head in "transposed space": S^T[k,q] = K-block @ Q^T,
     exp on ACT, causal corner mask via a triangle multiply on DVE, then
     out^T = [V|1].T @ A_un^T, software-pipelined so the PE never waits on
     the exp of the current block. The appended ones-column makes the
     softmax denominators fall out of the same matmul (PSUM row 64);
     normalization is a tiny reciprocal + DMA broadcast per head.
  4. AllToAll O^T piece exchange, then c_proj of the core's 512-row slice.

Attention runs largest chunk first so the tail chunk is the cheapest.
"""

import sys

sys.path.insert(0, "/opt/trn_rl_repo")

import numpy as np

import concourse.bass as bass
import concourse.mybir as mybir
import concourse.tile as tile
from concourse import bacc
from concourse.bass_utils import run_bass_kernel_spmd
from concourse.masks import make_identity

B, S, H, NH, DK = 2, 2048, 1024, 16, 64
NCORES = 8
HPC = 4            # heads per core
CW = HPC * DK      # 256 qkv columns per core
SLICE = S // 4     # 512 output rows per core
GROUPS = [[0, 1, 2, 3], [4, 5, 6, 7]]

F32 = mybir.dt.float32
DT_MM = mybir.dt.float32r  # matmul operand dtype: float32 | float32r


def _bcast_ap(src_ap, parts):
    """Partition-broadcast view: repeat src_ap's single row across `parts`."""
    ap = [list(p) for p in src_ap.ap]
    if len(ap) > 1 and ap[0][1] == 1:
        ap = ap[1:]  # drop singleton partition dim
    return bass.AP(
        tensor=src_ap.tensor,
        offset=src_ap.offset,
        ap=[[0, parts]] + ap,
    )


def build_nc(dt_mm=DT_MM):
    nc = bacc.Bacc(None, target_bir_lowering=False, debug=False, num_devices=NCORES)

    xb = nc.declare_dram_parameter("xb", [S, H], dt_mm, isOutput=False)
    wq = nc.declare_dram_parameter("wq", [H, CW], dt_mm, isOutput=False)
    wk = nc.declare_dram_parameter("wk", [H, CW], dt_mm, isOutput=False)
    wv = nc.declare_dram_parameter("wv", [H, CW], dt_mm, isOutput=False)
    wp = nc.declare_dram_parameter("wp", [H, H], dt_mm, isOutput=False)
    qoff = nc.declare_dram_parameter("qoff", [1, 1], mybir.dt.int32, isOutput=False)
    bq = nc.declare_dram_parameter("bq", [CW], F32, isOutput=False)
    bk = nc.declare_dram_parameter("bk", [CW], F32, isOutput=False)
    bv = nc.declare_dram_parameter("bv", [CW], F32, isOutput=False)
    out = nc.declare_dram_parameter("out", [SLICE, H], F32, isOutput=True)

    KT = H // 128   # 8 contraction tiles over H
    ST = S // 128   # 16 seq tiles
    NQ = S // 512   # 4 query chunks of 512

    with tile.TileContext(nc) as tc:
        with (
            tc.tile_pool(name="dram", bufs=1, space="DRAM") as dram,
            tc.tile_pool(name="psum", bufs=1, space="PSUM") as psum,
            tc.tile_pool(name="persist", bufs=1) as pw,
        ):
            ag_in = dram.tile([NQ, CW, 512], dt_mm)   # my heads' O^T, per chunk
            gathered = dram.tile([NQ * 4 * CW, 512], dt_mm)  # [chunk, all heads]

            ident = pw.tile([128, 128], dt_mm)
            ident_f32 = pw.tile([128, 128], F32)
            make_identity(nc, ident_f32)
            nc.vector.tensor_copy(ident, ident_f32)
            ones4 = pw.tile([128, HPC, 1], F32)
            nc.gpsimd.memset(ones4, 1.0)
            # lower-triangle-in-q mask: tri[k, q] = 1 if q >= k else 0
            tri_f32 = pw.tile([128, 128], F32)
            nc.gpsimd.memset(tri_f32, 1.0)
            nc.gpsimd.affine_select(
                out=tri_f32, in_=tri_f32, compare_op=mybir.AluOpType.is_ge,
                fill=0.0, base=0, pattern=[[1, 128]], channel_multiplier=-1)
            tri = pw.tile([128, 128], dt_mm)
            nc.vector.tensor_copy(tri, tri_f32)

            # qkv weights: [128, k-tile, cols]
            wq_sb = pw.tile([128, KT, CW], dt_mm)
            wk_sb = pw.tile([128, KT, CW], dt_mm)
            wv_sb = pw.tile([128, KT, CW], dt_mm)
            nc.gpsimd.dma_start(out=wq_sb, in_=wq.ap().rearrange("(k p) c -> p k c", p=128))
            nc.gpsimd.dma_start(out=wk_sb, in_=wk.ap().rearrange("(k p) c -> p k c", p=128))
            nc.gpsimd.dma_start(out=wv_sb, in_=wv.ap().rearrange("(k p) c -> p k c", p=128))

            # biases
            bq_sb = pw.tile([128, 2], F32)
            bk_sb = pw.tile([128, 2], F32)
            nc.gpsimd.dma_start(out=bq_sb, in_=bq.ap().rearrange("(h p) -> p h", p=128))
            nc.gpsimd.dma_start(out=bk_sb, in_=bk.ap().rearrange("(h p) -> p h", p=128))
            bv_bc = pw.tile([128, CW], F32)
            nc.gpsimd.dma_start(out=bv_bc, in_=_bcast_ap(bv.ap(), 128))

            QTt = pw.tile([128, 2, S], dt_mm)   # q-col (128) x [half, s]
            KTt = pw.tile([128, 2, S], dt_mm)
            V4 = pw.tile([128, ST, HPC, DK + 1], dt_mm)  # [s-part, s-tile, head, dk|1]

            with tc.tile_pool(name="px", bufs=1) as px:
                xT = px.tile([128, KT, S], dt_mm)   # h-part x [h-tile, s]

                # ---- phase 0: x^T ----
                for si in range(ST):
                    xs = px.tile([128, H], dt_mm, tag="xs", bufs=2)
                    nc.sync.dma_start(out=xs, in_=xb[si * 128:(si + 1) * 128, :])
                    for k in range(KT):
                        pt = psum.tile([128, 128], dt_mm, tag="tpav", bufs=3)
                        nc.tensor.transpose(pt, xs[:, k * 128:(k + 1) * 128], ident)
                        nc.vector.tensor_copy(xT[:, k, si * 128:(si + 1) * 128], pt)

                # ---- phase 1: all QKV ----
                for j in range(NQ):
                    js = slice(j * 512, (j + 1) * 512)
                    for (wt, dst, bias) in ((wq_sb, QTt, bq_sb), (wk_sb, KTt, bk_sb)):
                        for half in range(2):
                            pq = psum.tile([128, 512], F32, tag="qkv", bufs=2)
                            for k in range(KT):
                                nc.tensor.matmul(
                                    pq,
                                    wt[:, k, half * 128:(half + 1) * 128],
                                    xT[:, k, js],
                                    start=(k == 0),
                                    stop=(k == KT - 1),
                                )
                            nc.scalar.activation(
                                dst[:, half, js], pq,
                                mybir.ActivationFunctionType.Identity,
                                bias=bias[:, half:half + 1],
                            )
                    for si in range(4 * j, 4 * j + 4):
                        pv = psum.tile([128, CW], F32, tag="qkv", bufs=2)
                        for k in range(KT):
                            nc.tensor.matmul(
                                pv, xT[:, k, si * 128:(si + 1) * 128], wv_sb[:, k, :],
                                start=(k == 0), stop=(k == KT - 1),
                            )
                        pv_h = pv[:, :].rearrange("p (h d) -> p h d", h=HPC)
                        bv_h = bv_bc[:, :].rearrange("p (h d) -> p h d", h=HPC)
                        nc.vector.tensor_add(V4[:, si, :, 0:DK], pv_h, bv_h)
                        nc.vector.tensor_copy(V4[:, si, :, DK:DK + 1], ones4)

            # ---- phases 2+3 (px closed: its SBUF is reused below) ----
            with tc.tile_pool(name="pproj", bufs=1) as pproj:
                wp_sb = pproj.tile([128, KT, H], dt_mm)
                nc.gpsimd.dma_start(
                    out=wp_sb, in_=wp.ap().rearrange("(k p) c -> p k c", p=128))

                # attention chunks, largest first (cheap chunk at the tail)
                for j in range(NQ):
                    for h in range(HPC):
                        pb = 64 * (h % 2)
                        qt = QTt[pb:pb + DK, h // 2, :]
                        kt = KTt[pb:pb + DK, h // 2, :]
                        comb = pproj.tile([65, 512], dt_mm, tag="comb", bufs=6,
                                          name=f"comb{j}_{h}")
                        pav = psum.tile([65, 512], F32, tag="tpav", bufs=3)
                        nblk = 4 * j + 4
                        # software-pipelined: S^T(ki) on the PE while
                        # exp/mask of ki-1 are in flight, then AV(ki-1)
                        As = {}
                        offs = {}
                        for ki in range(nblk):
                            off = max(0, 128 * ki - 512 * j)
                            npp = 512 - off
                            offs[ki] = (off, npp)
                            ps = psum.tile([128, 512], F32, tag="sT", bufs=3)
                            nc.tensor.matmul(
                                ps[:, :npp],
                                kt[:, ki * 128:(ki + 1) * 128],
                                qt[:, j * 512 + off:(j + 1) * 512],
                                start=True, stop=True,
                            )
                            A = pproj.tile([128, 512], dt_mm, tag="A", bufs=5,
                                           name=f"A{j}_{h}_{ki}")
                            nc.scalar.activation(
                                A[:, :npp], ps[:, :npp],
                                mybir.ActivationFunctionType.Exp)
                            if ki >= 4 * j:
                                # corner: zero the k > q triangle
                                nc.vector.tensor_mul(A[:, :128], A[:, :128], tri)
                            As[ki] = A
                            if ki >= 1:
                                o, n_ = offs[ki - 1]
                                nc.tensor.matmul(
                                    pav[:, o:], V4[:, ki - 1, h, :],
                                    As.pop(ki - 1)[:, :n_],
                                    start=(ki - 1 == 0), stop=False,
                                )
                        o, n_ = offs[nblk - 1]
                        nc.tensor.matmul(
                            pav[:, o:], V4[:, nblk - 1, h, :],
                            As.pop(nblk - 1)[:, :n_],
                            start=(nblk == 1), stop=True,
                        )
                        nc.vector.tensor_copy(comb, pav)

                        # per-head normalization, pipelined across heads:
                        # sums -> [128,4] reciprocal -> broadcast -> scale
                        sums_h = dram.tile([512], dt_mm, name=f"sums{j}_{h}")
                        rsums_h = dram.tile([512], dt_mm, name=f"rsums{j}_{h}")
                        nc.sync.dma_start(out=sums_h[:], in_=comb[64:65, :])
                        sre = pproj.tile([128, 4], dt_mm, tag="sre", bufs=4,
                                         name=f"sre{j}_{h}")
                        nc.sync.dma_start(
                            out=sre, in_=sums_h[:].rearrange("(p f) -> p f", p=128))
                        with nc.allow_low_precision(
                                reason="f32r recip of O(1e3) softmax sums"):
                            nc.vector.reciprocal(sre, sre)
                        nc.sync.dma_start(
                            out=rsums_h[:].rearrange("(p f) -> p f", p=128), in_=sre)
                        rbc = pproj.tile([64, 512], dt_mm, tag="rbc", bufs=3,
                                         name=f"rbc{j}_{h}")
                        nc.sync.dma_start(out=rbc, in_=_bcast_ap(rsums_h[:], 64))
                        nc.vector.tensor_mul(comb[0:64, :], comb[0:64, :], rbc)
                        # pack into this chunk's AllGather input
                        nc.sync.dma_start(
                            out=ag_in[j, 64 * h:64 * h + 64, :],
                            in_=comb[0:64, :])
                    nc.gpsimd.collective_compute(
                        "AllGather",
                        mybir.AluOpType.bypass,
                        replica_groups=GROUPS,
                        ins=[ag_in[j].opt()],
                        outs=[gathered[1024 * j:1024 * (j + 1), :].opt()],
                    )

                # ---- c_proj of my 512-row slice, full head dimension ----
                qoff_sb = pproj.tile([1, 1], mybir.dt.int32)
                nc.gpsimd.dma_start(out=qoff_sb, in_=qoff[:, :])
                og_sb = pproj.tile([128, KT, SLICE], dt_mm)
                with nc.gpsimd.register("qor") as qor:
                    nc.gpsimd.load(qor, qoff_sb[0:1, 0:1])
                    qsnap = nc.gpsimd.snap(qor)
                    nc.gpsimd.dma_start(
                        out=og_sb,
                        in_=gathered[bass.ds(qsnap, 4 * CW), :].rearrange(
                            "(k p) q -> p k q", p=128))
                for t in range(4):
                    yt = pproj.tile([128, H], F32, tag="y", bufs=2, name=f"yt{t}")
                    for n in range(2):
                        py = psum.tile([128, 512], F32, tag="qkv", bufs=2,
                                       name=f"py{t}_{n}")
                        for k in range(KT):
                            nc.tensor.matmul(
                                py,
                                og_sb[:, k, t * 128:(t + 1) * 128],
                                wp_sb[:, k, n * 512:(n + 1) * 512],
                                start=(k == 0), stop=(k == KT - 1),
                            )
                        nc.vector.tensor_copy(yt[:, n * 512:(n + 1) * 512], py)
                    nc.sync.dma_start(out=out[t * 128:(t + 1) * 128, :], in_=yt)

    nc.compile()
    return nc


_NC = None


def kernel(x, w_attn, b_attn, w_proj, b_proj):
    global _NC
    if _NC is None:
        _NC = build_nc()

    x = np.ascontiguousarray(np.asarray(x, dtype=np.float32))
    w_attn = np.asarray(w_attn, dtype=np.float32)
    b_attn = np.asarray(b_attn, dtype=np.float32)
    w_proj = np.ascontiguousarray(np.asarray(w_proj, dtype=np.float32))
    b_proj = np.asarray(b_proj, dtype=np.float32)

    in_maps = []
    for c in range(NCORES):
        b, g = divmod(c, 4)
        h0 = g * HPC
        cs = slice(h0 * DK, h0 * DK + CW)
        in_maps.append({
            "xb": x[b],
            # fold the 1/sqrt(DK)=2^-3 score scale into Wq/bq (exact in fp32)
            "wq": np.ascontiguousarray(w_attn[:, cs]) * np.float32(0.125),
            "wk": np.ascontiguousarray(w_attn[:, H:][:, cs]),
            "wv": np.ascontiguousarray(w_attn[:, 2 * H:][:, cs]),
            "wp": w_proj,
            "qoff": np.array([[g * 4 * CW]], dtype=np.int32),
            "bq": np.ascontiguousarray(b_attn[cs]) * np.float32(0.125),
            "bk": np.ascontiguousarray(b_attn[H:][cs]),
            "bv": np.ascontiguousarray(b_attn[2 * H:][cs]),
        })

    res = run_bass_kernel_spmd(_NC, in_maps, core_ids=list(range(NCORES)))

    outp = np.empty((B, S, H), dtype=np.float32)
    for c in range(NCORES):
        b, g = divmod(c, 4)
        outp[b, g * SLICE:(g + 1) * SLICE, :] = res.results[c]["out"]
    outp += b_proj  # row-broadcast add, exact
    return outp


# revision 27
# speedup vs baseline: 1.0584x; 1.0584x over previous
"""Causal self-attention (B=2, S=2048, H=1024, NH=16) on 8 TRN2 NeuronCores.

Sharding: core c handles batch b = c//4 and heads [4*(c%4), 4*(c%4)+4).
Tensor-parallel c_attn (column split); the output projection is computed
from the full head dimension on a 512-row output slice per core, after
per-chunk AllGathers inside each 4-core batch group redistribute O^T.
(This moves 4x less data than reduce-scattering partial c_proj outputs and
cuts per-core c_proj flops 4x; the per-rank column slice is selected with
a data-driven register offset so the program stays SPMD-uniform.)

Per-core dataflow (all matmuls on the PE array):
  1. x[b] -> x^T via PE transposes (contraction over H needs H on partitions).
  2. QKV: Q^T,K^T [dk, S] and V [S, dk] for its 4 heads (Wq pre-scaled 1/8).
  3. Attention per head in "transposed space": S^T[k,q] = K-block @ Q^T,
     exp on ACT, causal corner mask via a triangle multiply on DVE, then
     out^T = [V|1].T @ A_un^T, software-pipelined so the PE never waits on
     the exp of the current block. The appended ones-column makes the
     softmax denominators fall out of the same matmul (PSUM row 64);
     normalization is a tiny reciprocal + DMA broadcast per head.
  4. Per-chunk AllGather of O^T pieces (overlapped with later chunks'
     attention), then c_proj of the core's 512-row slice.
"""

import sys

sys.path.insert(0, "/opt/trn_rl_repo")

import numpy as np

import concourse.bass as bass
import concourse.mybir as mybir
import concourse.tile as tile
from concourse import bacc
from concourse.bass_utils import run_bass_kernel_spmd
from concourse.masks import make_identity

B, S, H, NH, DK = 2, 2048, 1024, 16, 64
NCORES = 8
HPC = 4            # heads per core
CW = HPC * DK      # 256 qkv columns per core
SLICE = S // 4     # 512 output rows per core
GROUPS = [[0, 1, 2, 3], [4, 5, 6, 7]]

F32 = mybir.dt.float32
DT_MM = mybir.dt.float32r  # matmul operand dtype: float32 | float32r


def _bcast_ap(src_ap, parts):
    """Partition-broadcast view: repeat src_ap's single row across `parts`."""
    ap = [list(p) for p in src_ap.ap]
    if len(ap) > 1 and ap[0][1] == 1:
        ap = ap[1:]  # drop singleton partition dim
    return bass.AP(
        tensor=src_ap.tensor,
        offset=src_ap.offset,
        ap=[[0, parts]] + ap,
    )


def build_nc(dt_mm=DT_MM):
    nc = bacc.Bacc(None, target_bir_lowering=False, debug=False, num_devices=NCORES)

    xb = nc.declare_dram_parameter("xb", [S, H], dt_mm, isOutput=False)
    wq = nc.declare_dram_parameter("wq", [H, CW], dt_mm, isOutput=False)
    wk = nc.declare_dram_parameter("wk", [H, CW], dt_mm, isOutput=False)
    wv = nc.declare_dram_parameter("wv", [H, CW], dt_mm, isOutput=False)
    wp = nc.declare_dram_parameter("wp", [H, H], dt_mm, isOutput=False)
    qoff = nc.declare_dram_parameter("qoff", [1, 1], mybir.dt.int32, isOutput=False)
    bq = nc.declare_dram_parameter("bq", [CW], F32, isOutput=False)
    bk = nc.declare_dram_parameter("bk", [CW], F32, isOutput=False)
    bv = nc.declare_dram_parameter("bv", [CW], F32, isOutput=False)
    out = nc.declare_dram_parameter("out", [SLICE, H], F32, isOutput=True)

    KT = H // 128   # 8 contraction tiles over H
    ST = S // 128   # 16 seq tiles
    NQ = S // 512   # 4 query chunks of 512

    with tile.TileContext(nc) as tc:
        with (
            tc.tile_pool(name="dram", bufs=1, space="DRAM") as dram,
            tc.tile_pool(name="psum", bufs=1, space="PSUM") as psum,
            tc.tile_pool(name="persist", bufs=1) as pw,
        ):
            ag_in = dram.tile([NQ, CW, 512], dt_mm)   # my heads' O^T, per chunk
            gathered = dram.tile([NQ * 4 * CW, 512], dt_mm)  # [chunk, all heads]

            ident = pw.tile([128, 128], dt_mm)
            ident_f32 = pw.tile([128, 128], F32)
            make_identity(nc, ident_f32)
            nc.vector.tensor_copy(ident, ident_f32)
            ones4 = pw.tile([128, HPC, 1], F32)
            nc.gpsimd.memset(ones4, 1.0)
            # lower-triangle-in-q mask: tri[k, q] = 1 if q >= k else 0
            tri_f32 = pw.tile([128, 128], F32)
            nc.gpsimd.memset(tri_f32, 1.0)
            nc.gpsimd.affine_select(
                out=tri_f32, in_=tri_f32, compare_op=mybir.AluOpType.is_ge,
                fill=0.0, base=0, pattern=[[1, 128]], channel_multiplier=-1)
            tri = pw.tile([128, 128], dt_mm)
            nc.vector.tensor_copy(tri, tri_f32)

            # qkv weights: [128, k-tile, cols]
            wq_sb = pw.tile([128, KT, CW], dt_mm)
            wk_sb = pw.tile([128, KT, CW], dt_mm)
            wv_sb = pw.tile([128, KT, CW], dt_mm)
            nc.gpsimd.dma_start(out=wq_sb, in_=wq.ap().rearrange("(k p) c -> p k c", p=128))
            nc.gpsimd.dma_start(out=wk_sb, in_=wk.ap().rearrange("(k p) c -> p k c", p=128))
            nc.gpsimd.dma_start(out=wv_sb, in_=wv.ap().rearrange("(k p) c -> p k c", p=128))

            # biases
            bq_sb = pw.tile([128, 2], F32)
            bk_sb = pw.tile([128, 2], F32)
            nc.gpsimd.dma_start(out=bq_sb, in_=bq.ap().rearrange("(h p) -> p h", p=128))
            nc.gpsimd.dma_start(out=bk_sb, in_=bk.ap().rearrange("(h p) -> p h", p=128))
            bv_bc = pw.tile([128, CW], F32)
            nc.gpsimd.dma_start(out=bv_bc, in_=_bcast_ap(bv.ap(), 128))

            QTt = pw.tile([128, 2, S], dt_mm)   # q-col (128) x [half, s]
            KTt = pw.tile([128, 2, S], dt_mm)
            V4 = pw.tile([128, ST, HPC, DK + 1], dt_mm)  # [s-part, s-tile, head, dk|1]

            with tc.tile_pool(name="px", bufs=1) as px:
                xT = px.tile([128, KT, S], dt_mm)   # h-part x [h-tile, s]

                # ---- phase 0: x^T ----
                for si in range(ST):
                    xs = px.tile([128, H], dt_mm, tag="xs", bufs=2)
                    nc.sync.dma_start(out=xs, in_=xb[si * 128:(si + 1) * 128, :])
                    for k in range(KT):
                        pt = psum.tile([128, 128], dt_mm, tag="tpav", bufs=3)
                        nc.tensor.transpose(pt, xs[:, k * 128:(k + 1) * 128], ident)
                        nc.vector.tensor_copy(xT[:, k, si * 128:(si + 1) * 128], pt)

                # ---- phase 1: all QKV ----
                for j in range(NQ):
                    js = slice(j * 512, (j + 1) * 512)
                    for (wt, dst, bias) in ((wq_sb, QTt, bq_sb), (wk_sb, KTt, bk_sb)):
                        for half in range(2):
                            pq = psum.tile([128, 512], F32, tag="qkv", bufs=2)
                            for k in range(KT):
                                nc.tensor.matmul(
                                    pq,
                                    wt[:, k, half * 128:(half + 1) * 128],
                                    xT[:, k, js],
                                    start=(k == 0),
                                    stop=(k == KT - 1),
                                )
                            nc.scalar.activation(
                                dst[:, half, js], pq,
                                mybir.ActivationFunctionType.Identity,
                                bias=bias[:, half:half + 1],
                            )
                    for si in range(4 * j, 4 * j + 4):
                        pv = psum.tile([128, CW], F32, tag="qkv", bufs=2)
                        for k in range(KT):
                            nc.tensor.matmul(
                                pv, xT[:, k, si * 128:(si + 1) * 128], wv_sb[:, k, :],
                                start=(k == 0), stop=(k == KT - 1),
                            )
                        pv_h = pv[:, :].rearrange("p (h d) -> p h d", h=HPC)
                        bv_h = bv_bc[:, :].rearrange("p (h d) -> p h d", h=HPC)
                        nc.vector.tensor_add(V4[:, si, :, 0:DK], pv_h, bv_h)
                        nc.vector.tensor_copy(V4[:, si, :, DK:DK + 1], ones4)

            # ---- phases 2+3 (px closed: its SBUF is reused below) ----
            with tc.tile_pool(name="pproj", bufs=1) as pproj:
                wp_sb = pproj.tile([128, KT, H], dt_mm)
                nc.gpsimd.dma_start(
                    out=wp_sb, in_=wp.ap().rearrange("(k p) c -> p k c", p=128))

                # attention chunks, largest first (cheap chunk at the tail)
                for j in range(NQ):
                    for h in range(HPC):
                        pb = 64 * (h % 2)
                        qt = QTt[pb:pb + DK, h // 2, :]
                        kt = KTt[pb:pb + DK, h // 2, :]
                        comb = pproj.tile([65, 512], dt_mm, tag="comb", bufs=6,
                                          name=f"comb{j}_{h}")
                        pav = psum.tile([65, 512], F32, tag="tpav", bufs=3)
                        nblk = 4 * j + 4
                        # software-pipelined: S^T(ki) on the PE while
                        # exp/mask of ki-1 are in flight, then AV(ki-1)
                        As = {}
                        offs = {}
                        for ki in range(nblk):
                            off = max(0, 128 * ki - 512 * j)
                            npp = 512 - off
                            offs[ki] = (off, npp)
                            ps = psum.tile([128, 512], F32, tag="sT", bufs=3)
                            nc.tensor.matmul(
                                ps[:, :npp],
                                kt[:, ki * 128:(ki + 1) * 128],
                                qt[:, j * 512 + off:(j + 1) * 512],
                                start=True, stop=True,
                            )
                            A = pproj.tile([128, 512], dt_mm, tag="A", bufs=5,
                                           name=f"A{j}_{h}_{ki}")
                            nc.scalar.activation(
                                A[:, :npp], ps[:, :npp],
                                mybir.ActivationFunctionType.Exp)
                            if ki >= 4 * j:
                                # corner: zero the k > q triangle
                                nc.vector.tensor_mul(A[:, :128], A[:, :128], tri)
                            As[ki] = A
                            if ki >= 1:
                                o, n_ = offs[ki - 1]
                                nc.tensor.matmul(
                                    pav[:, o:], V4[:, ki - 1, h, :],
                                    As.pop(ki - 1)[:, :n_],
                                    start=(ki - 1 == 0), stop=False,
                                )
                        o, n_ = offs[nblk - 1]
                        nc.tensor.matmul(
                            pav[:, o:], V4[:, nblk - 1, h, :],
                            As.pop(nblk - 1)[:, :n_],
                            start=(nblk == 1), stop=True,
                        )
                        nc.vector.tensor_copy(comb, pav)

                        # per-head normalization, pipelined across heads:
                        # sums -> [128,4] reciprocal -> broadcast -> scale
                        sums_h = dram.tile([512], dt_mm, name=f"sums{j}_{h}")
                        rsums_h = dram.tile([512], dt_mm, name=f"rsums{j}_{h}")
                        nc.sync.dma_start(out=sums_h[:], in_=comb[64:65, :])
                        sre = pproj.tile([128, 4], dt_mm, tag="sre", bufs=4,
                                         name=f"sre{j}_{h}")
                        nc.sync.dma_start(
                            out=sre, in_=sums_h[:].rearrange("(p f) -> p f", p=128))
                        with nc.allow_low_precision(
                                reason="f32r recip of O(1e3) softmax sums"):
                            nc.vector.reciprocal(sre, sre)
                        nc.sync.dma_start(
                            out=rsums_h[:].rearrange("(p f) -> p f", p=128), in_=sre)
                        rbc = pproj.tile([64, 512], dt_mm, tag="rbc", bufs=3,
                                         name=f"rbc{j}_{h}")
                        nc.gpsimd.dma_start(out=rbc, in_=_bcast_ap(rsums_h[:], 64))
                        nc.vector.tensor_mul(comb[0:64, :], comb[0:64, :], rbc)
                        # pack into this chunk's AllGather input
                        nc.sync.dma_start(
                            out=ag_in[j, 64 * h:64 * h + 64, :],
                            in_=comb[0:64, :])
                    nc.gpsimd.collective_compute(
                        "AllGather",
                        mybir.AluOpType.bypass,
                        replica_groups=GROUPS,
                        ins=[ag_in[j].opt()],
                        outs=[gathered[1024 * j:1024 * (j + 1), :].opt()],
                    )

                # ---- c_proj of my 512-row slice, full head dimension ----
                qoff_sb = pproj.tile([1, 1], mybir.dt.int32)
                nc.gpsimd.dma_start(out=qoff_sb, in_=qoff[:, :])
                og_sb = pproj.tile([128, KT, SLICE], dt_mm)
                with nc.gpsimd.register("qor") as qor:
                    nc.gpsimd.load(qor, qoff_sb[0:1, 0:1])
                    qsnap = nc.gpsimd.snap(qor)
                    nc.gpsimd.dma_start(
                        out=og_sb,
                        in_=gathered[bass.ds(qsnap, 4 * CW), :].rearrange(
                            "(k p) q -> p k q", p=128))
                for t in range(4):
                    yt = pproj.tile([128, H], F32, tag="y", bufs=2, name=f"yt{t}")
                    for n in range(2):
                        py = psum.tile([128, 512], F32, tag="qkv", bufs=2,
                                       name=f"py{t}_{n}")
                        for k in range(KT):
                            nc.tensor.matmul(
                                py,
                                og_sb[:, k, t * 128:(t + 1) * 128],
                                wp_sb[:, k, n * 512:(n + 1) * 512],
                                start=(k == 0), stop=(k == KT - 1),
                            )
                        nc.vector.tensor_copy(yt[:, n * 512:(n + 1) * 512], py)
                    nc.sync.dma_start(out=out[t * 128:(t + 1) * 128, :], in_=yt)

    nc.compile()
    return nc


_NC = None


def kernel(x, w_attn, b_attn, w_proj, b_proj):
    global _NC
    if _NC is None:
        _NC = build_nc()

    x = np.ascontiguousarray(np.asarray(x, dtype=np.float32))
    w_attn = np.asarray(w_attn, dtype=np.float32)
    b_attn = np.asarray(b_attn, dtype=np.float32)
    w_proj = np.ascontiguousarray(np.asarray(w_proj, dtype=np.float32))
    b_proj = np.asarray(b_proj, dtype=np.float32)

    in_maps = []
    for c in range(NCORES):
        b, g = divmod(c, 4)
        h0 = g * HPC
        cs = slice(h0 * DK, h0 * DK + CW)
        in_maps.append({
            "xb": x[b],
            # fold the 1/sqrt(DK)=2^-3 score scale into Wq/bq (exact in fp32)
            "wq": np.ascontiguousarray(w_attn[:, cs]) * np.float32(0.125),
            "wk": np.ascontiguousarray(w_attn[:, H:][:, cs]),
            "wv": np.ascontiguousarray(w_attn[:, 2 * H:][:, cs]),
            "wp": w_proj,
            "qoff": np.array([[g * 4 * CW]], dtype=np.int32),
            "bq": np.ascontiguousarray(b_attn[cs]) * np.float32(0.125),
            "bk": np.ascontiguousarray(b_attn[H:][cs]),
            "bv": np.ascontiguousarray(b_attn[2 * H:][cs]),
        })

    res = run_bass_kernel_spmd(_NC, in_maps, core_ids=list(range(NCORES)))

    outp = np.empty((B, S, H), dtype=np.float32)
    for c in range(NCORES):
        b, g = divmod(c, 4)
        outp[b, g * SLICE:(g + 1) * SLICE, :] = res.results[c]["out"]
    outp += b_proj  # row-broadcast add, exact
    return outp
